# revision 5
# baseline (speedup 1.0000x reference)
"""FP8 Linear (dynamic per-tensor e4m3 quantization) on 8 Trainium2 NeuronCores.

Computes the forward value of:
    x_q, s_x = quantize_e4m3fn(x);  w_q, s_w = quantize_e4m3fn(weight)
    out = bf16((x_q*s_x) @ (w_q*s_w).T)        # the bf16 STE shadow GEMM is a
                                               # forward no-op up to bf16 rounding
Strategy per core (data-parallel over tokens, weight replicated):
  1. abs-max over the local x shard + over (full, replicated) weight.
  2. AllReduce(max) of the x amax across the 8 cores (weight amax needs none).
  3. Quantize to the *Trainium* fp8-e4m3 grid at HALF the reference scale
     (TRN e4m3 max normal is 240, not 448; the e4m3fn grid divided by 2 is
     exactly representable, so rounding commutes) and rescale the GEMM output
     by 4*s_x*s_w.
  4. Tiled fp8 GEMM (optionally DoubleRow 2x-pumped), fp32 PSUM accumulation,
     fused scale+bf16-cast on the ScalarE PSUM drain.
"""

import numpy as np
import ml_dtypes

TOKENS, D_IN, D_OUT = 32768, 2048, 2048
N_CORES = 8
USE_DOUBLE_ROW = True

_BUILD_CACHE = {}
LAST_RESULT = None  # BassKernelResults of the most recent device run
TRACE = False       # set True (e.g. from test.py) to capture an NTFF profile


def _build(tok_per_core, d_in, d_out, n_cores, use_double_row, mblk):
    key = (tok_per_core, d_in, d_out, n_cores, use_double_row, mblk)
    if key in _BUILD_CACHE:
        return _BUILD_CACHE[key]

    import concourse.bass as bass
    import concourse.mybir as mybir
    import concourse.tile as tile
    from concourse import bacc, bass_isa

    DT = mybir.dt
    P = 128
    NTILE = min(512, d_out)
    assert tok_per_core % mblk == 0 and mblk % P == 0
    assert d_in % P == 0 and d_out % NTILE == 0 and d_out % P == 0
    KT = d_in // P            # k-tiles of 128 along in_features
    NBLK = tok_per_core // mblk
    MSUB = mblk // P          # 128-row m-subtiles per block
    NNT = d_out // NTILE      # 512-wide n-tiles
    NXT = tok_per_core // P   # natural x tiles for the amax pass
    NWT = d_out // P          # natural w tiles for the amax pass
    if use_double_row:
        assert KT % 2 == 0

    nc = bacc.Bacc(None, target_bir_lowering=False, num_devices=n_cores)
    xs = nc.dram_tensor("xs", [tok_per_core, d_in], DT.bfloat16, kind="ExternalInput")
    w = nc.dram_tensor("w", [d_out, d_in], DT.bfloat16, kind="ExternalInput")
    out = nc.dram_tensor("out", [tok_per_core, d_out], DT.bfloat16, kind="ExternalOutput")

    F32 = DT.float32
    FP8 = DT.float8e4

    with tile.TileContext(nc) as tc:
        with (
            tc.tile_pool(name="consts", bufs=1) as cpool,
            tc.tile_pool(name="nat", bufs=4) as natpool,
            tc.tile_pool(name="tp", bufs=4) as tppool,
            tc.tile_pool(name="xq", bufs=2) as xqpool,
            tc.tile_pool(name="ob", bufs=3) as obpool,
            tc.tile_pool(name="ps", bufs=8, space="PSUM") as pspool,
            tc.tile_pool(name="dram", bufs=1, space="DRAM") as dpool,
        ):
            # ---------------- phase 1: abs-max scan (natural layout) -------------
            pmax = cpool.tile([P, NXT + NWT], F32)
            for i in range(NXT):
                xt = natpool.tile([P, d_in], DT.bfloat16, tag="nat")
                nc.scalar.dma_start(xt, xs[i * P:(i + 1) * P, :])
                nc.vector.tensor_reduce(
                    pmax[:, i:i + 1], xt, axis=mybir.AxisListType.X,
                    op=mybir.AluOpType.max, apply_absolute_value=True)
            for j in range(NWT):
                wt = natpool.tile([P, d_in], DT.bfloat16, tag="nat")
                nc.scalar.dma_start(wt, w[j * P:(j + 1) * P, :])
                nc.vector.tensor_reduce(
                    pmax[:, NXT + j:NXT + j + 1], wt, axis=mybir.AxisListType.X,
                    op=mybir.AluOpType.max, apply_absolute_value=True)

            lmax = cpool.tile([P, 2], F32)
            nc.vector.tensor_reduce(lmax[:, 0:1], pmax[:, 0:NXT],
                                    axis=mybir.AxisListType.X, op=mybir.AluOpType.max)
            nc.vector.tensor_reduce(lmax[:, 1:2], pmax[:, NXT:NXT + NWT],
                                    axis=mybir.AxisListType.X, op=mybir.AluOpType.max)
            gmax = cpool.tile([P, 2], F32)
            nc.gpsimd.partition_all_reduce(gmax, lmax, channels=P,
                                           reduce_op=bass_isa.ReduceOp.max)
            # gmax[:,0] = local-shard amax(x), gmax[:,1] = amax(w) (already global)

            # ------------- cross-core AllReduce(max) of the x amax ---------------
            cc_in = dpool.tile([1, 1], F32)
            cc_out = dpool.tile([1, 1], F32,
                                addr_space="Shared" if n_cores > 4 else "Local")
            nc.gpsimd.dma_start(cc_in, gmax[0:1, 0:1])
            nc.gpsimd.collective_compute(
                "AllReduce", mybir.AluOpType.max,
                replica_groups=[list(range(n_cores))],
                ins=[cc_in.opt()], outs=[cc_out.opt()])
            ax1 = cpool.tile([1, 1], F32)
            nc.gpsimd.dma_start(ax1, cc_out)
            axb = cpool.tile([P, 1], F32)
            nc.gpsimd.partition_broadcast(axb, ax1)

            # ---------------- scales (per-partition [128,1] copies) --------------
            awb = cpool.tile([P, 1], F32)
            nc.vector.tensor_scalar_max(axb, axb, 1e-12)
            nc.vector.tensor_scalar_max(awb, gmax[:, 1:2], 1e-12)
            qsx = cpool.tile([P, 1], F32)   # 224/amax_x  (half the e4m3fn scale)
            qsw = cpool.tile([P, 1], F32)
            nc.vector.reciprocal(qsx, axb)
            nc.vector.tensor_scalar_mul(qsx, qsx, 224.0)
            nc.vector.reciprocal(qsw, awb)
            nc.vector.tensor_scalar_mul(qsw, qsw, 224.0)
            alpha = cpool.tile([P, 1], F32)  # 4*(ax/448)*(aw/448) = ax*aw/50176
            nc.vector.tensor_mul(alpha, axb, awb)
            nc.vector.tensor_scalar_mul(alpha, alpha, float(np.float32(1.0) / np.float32(50176.0)))

            # ------------- load w transposed, quantize to fp8 slab ---------------
            wq = cpool.tile([P, KT, d_out], FP8)
            for kt in range(KT):
                wtt = tppool.tile([P, d_out], DT.bfloat16, tag="tp")
                nc.sync.dma_start(wtt, w[:, kt * P:(kt + 1) * P], transpose=True)
                nc.vector.tensor_scalar_mul(wq[:, kt, :], wtt, qsw)

            # ---------------- phase 2: stream x^T, quantize, GEMM ----------------
            for blk in range(NBLK):
                xq = xqpool.tile([P, KT, mblk], FP8, tag="xq")
                for kt in range(KT):
                    xtt = tppool.tile([P, mblk], DT.bfloat16, tag="tp")
                    nc.sync.dma_start(
                        xtt, xs[blk * mblk:(blk + 1) * mblk, kt * P:(kt + 1) * P],
                        transpose=True)
                    nc.vector.tensor_scalar_mul(xq[:, kt, :], xtt, qsx)
                for mt in range(MSUB):
                    ob = obpool.tile([P, d_out], DT.bfloat16, tag="ob")
                    psums = []
                    for nt in range(NNT):
                        ps = pspool.tile([P, NTILE], F32, tag="ps")
                        psums.append(ps)
                    mlo = mt * P
                    if use_double_row:
                        for kp in range(KT // 2):
                            for nt in range(NNT):
                                nc.tensor.matmul(
                                    psums[nt],
                                    lhsT=xq[:, 2 * kp:2 * kp + 2, mlo:mlo + P],
                                    rhs=wq[:, 2 * kp:2 * kp + 2, nt * NTILE:(nt + 1) * NTILE],
                                    start=(kp == 0), stop=(kp == KT // 2 - 1),
                                    perf_mode=mybir.MatmulPerfMode.DoubleRow)
                    else:
                        for kt in range(KT):
                            for nt in range(NNT):
                                nc.tensor.matmul(
                                    psums[nt],
                                    lhsT=xq[:, kt, mlo:mlo + P],
                                    rhs=wq[:, kt, nt * NTILE:(nt + 1) * NTILE],
                                    start=(kt == 0), stop=(kt == KT - 1))
                    for nt in range(NNT):
                        nc.scalar.mul(ob[:, nt * NTILE:(nt + 1) * NTILE], psums[nt], alpha)
                    nc.scalar.dma_start(out[blk * mblk + mlo:blk * mblk + mlo + P, :], ob)

    nc.finalize()
    _BUILD_CACHE[key] = nc
    return nc


def kernel(x, weight):
    global LAST_RESULT
    from concourse.bass_utils import run_bass_kernel_spmd

    x = np.asarray(x)
    weight = np.asarray(weight)
    if x.dtype != ml_dtypes.bfloat16:
        x = x.astype(ml_dtypes.bfloat16)
    if weight.dtype != ml_dtypes.bfloat16:
        weight = weight.astype(ml_dtypes.bfloat16)
    assert x.shape == (TOKENS, D_IN) and weight.shape == (D_OUT, D_IN)

    tok = TOKENS // N_CORES
    nc = _build(tok, D_IN, D_OUT, N_CORES, USE_DOUBLE_ROW, mblk=2048)

    in_maps = [
        {"xs": np.ascontiguousarray(x[c * tok:(c + 1) * tok]), "w": weight}
        for c in range(N_CORES)
    ]
    res = run_bass_kernel_spmd(nc, in_maps, list(range(N_CORES)), trace=TRACE)
    LAST_RESULT = res
    return np.concatenate([res.results[c]["out"] for c in range(N_CORES)], axis=0)


# revision 11
# speedup vs baseline: 1.0401x; 1.0401x over previous
"""FP8 Linear (dynamic per-tensor e4m3 quantization) on 8 Trainium2 NeuronCores.

Computes the forward value of:
    x_q, s_x = quantize_e4m3fn(x);  w_q, s_w = quantize_e4m3fn(weight)
    out = bf16((x_q*s_x) @ (w_q*s_w).T)        # the bf16 STE shadow GEMM is a
                                               # forward no-op up to bf16 rounding
Strategy per core (data-parallel over tokens, weight replicated):
  1. abs-max over the local x shard (fused pairwise abs_max+reduce on DVE),
     AllReduce(max) across the 8 cores as early as possible; the (replicated)
     weight amax needs no collective and overlaps the AllReduce.
  2. Quantize to the *Trainium* fp8-e4m3 grid at HALF the reference scale
     (TRN e4m3 max normal is 240, not 448; the e4m3fn grid divided by 2 is
     exactly representable, so rounding commutes) and rescale the GEMM output
     by 4*s_x*s_w.
  3. Tiled fp8 GEMM (DoubleRow 2x-pumped), fp32 PSUM accumulation, fused
     scale+bf16-cast on the ScalarE PSUM drain.
  4. Dummy matmuls chained to phase-1 tiles keep the PE HAM clock at 2.4GHz
     through the amax/collective window; x^T DMA-transposes prefetch into
     SBUF during it.
"""

import numpy as np
import ml_dtypes

TOKENS, D_IN, D_OUT = 32768, 2048, 2048
N_CORES = 8
USE_DOUBLE_ROW = True
KEEP_WARM = True

_BUILD_CACHE = {}
LAST_RESULT = None  # BassKernelResults of the most recent device run
TRACE = False       # set True (e.g. from test.py) to capture an NTFF profile


def _build(tok_per_core, d_in, d_out, n_cores, use_double_row, mblk,
           keep_warm=KEEP_WARM):
    key = (tok_per_core, d_in, d_out, n_cores, use_double_row, mblk, keep_warm)
    if key in _BUILD_CACHE:
        return _BUILD_CACHE[key]

    import concourse.bass as bass
    import concourse.mybir as mybir
    import concourse.tile as tile
    from concourse import bacc, bass_isa

    DT = mybir.dt
    P = 128
    NTILE = min(512, d_out)
    assert tok_per_core % mblk == 0 and mblk % P == 0
    assert d_in % P == 0 and d_out % NTILE == 0 and d_out % P == 0
    KT = d_in // P            # k-tiles of 128 along in_features
    NBLK = tok_per_core // mblk
    MSUB = mblk // P          # 128-row m-subtiles per block
    NNT = d_out // NTILE      # n-tiles
    NXT = tok_per_core // P   # natural x tiles for the amax pass
    NWT = d_out // P          # natural w tiles for the amax pass
    WN = min(512, d_in)       # free dim for warm-up matmuls
    if use_double_row:
        assert KT % 2 == 0
    AMAX_CLAMP = 1e-12
    ALPHA_C = float(np.float32(1.0) / np.float32(50176.0))  # 4/448^2

    nc = bacc.Bacc(None, target_bir_lowering=False, num_devices=n_cores)
    xs = nc.dram_tensor("xs", [tok_per_core, d_in], DT.bfloat16, kind="ExternalInput")
    w = nc.dram_tensor("w", [d_out, d_in], DT.bfloat16, kind="ExternalInput")
    out = nc.dram_tensor("out", [tok_per_core, d_out], DT.bfloat16, kind="ExternalOutput")

    F32 = DT.float32
    FP8 = DT.float8e4
    AX = mybir.AxisListType.X
    MAX = mybir.AluOpType.max

    N_PRE = min(12, KT)  # x^T tiles of block 0 prefetched during phase 1

    with tile.TileContext(nc) as tc:
        with (
            tc.tile_pool(name="consts", bufs=1) as cpool,
            tc.tile_pool(name="nat", bufs=4) as natpool,
            tc.tile_pool(name="xtp", bufs=N_PRE) as xtpool,
            tc.tile_pool(name="wtp", bufs=3) as wtpool,
            tc.tile_pool(name="xq", bufs=2) as xqpool,
            tc.tile_pool(name="ob", bufs=2) as obpool,
            tc.tile_pool(name="ps", bufs=7, space="PSUM") as pspool,
            tc.tile_pool(name="warm", bufs=1, space="PSUM") as wmpool,
            tc.tile_pool(name="dram", bufs=1, space="DRAM") as dpool,
        ):
            warm_ps = wmpool.tile([P, NTILE], F32, name="warm_ps") if keep_warm else None

            def warm_mm(src_ap, dtype_is_fp8=False):
                if warm_ps is None:
                    return
                nc.tensor.matmul(warm_ps[:, 0:min(WN, NTILE)],
                                 lhsT=src_ap[:, 0:P],
                                 rhs=src_ap[:, 0:min(WN, NTILE)],
                                 start=True, stop=True)

            # ------------- phase 1a: x abs-max scan (natural layout) -------------
            pmax_x = cpool.tile([P, NXT], F32)
            for i in range(NXT):
                xt = natpool.tile([P, d_in], DT.bfloat16, tag="nat")
                nc.scalar.dma_start(xt, xs[i * P:(i + 1) * P, :])
                warm_mm(xt)
                nc.vector.tensor_reduce(
                    pmax_x[:, i:i + 1], xt, axis=AX, op=MAX,
                    apply_absolute_value=True)

            lx = cpool.tile([P, 1], F32)
            nc.vector.tensor_reduce(lx, pmax_x, axis=AX, op=MAX)
            gx = cpool.tile([P, 1], F32)
            nc.gpsimd.partition_all_reduce(gx, lx, channels=P,
                                           reduce_op=bass_isa.ReduceOp.max)

            # ------------- cross-core AllReduce(max) of the x amax ---------------
            cc_in = dpool.tile([1, 1], F32)
            cc_out = dpool.tile([1, 1], F32,
                                addr_space="Shared" if n_cores > 4 else "Local")
            nc.gpsimd.dma_start(cc_in, gx[0:1, 0:1])
            nc.gpsimd.collective_compute(
                "AllReduce", MAX, replica_groups=[list(range(n_cores))],
                ins=[cc_in.opt()], outs=[cc_out.opt()])
            ax1 = cpool.tile([1, 1], F32)
            nc.gpsimd.dma_start(ax1, cc_out)
            axb = cpool.tile([P, 1], F32)
            nc.gpsimd.partition_broadcast(axb, ax1)
            nc.vector.tensor_scalar_max(axb, axb, AMAX_CLAMP)
            qsx = cpool.tile([P, 1], F32)   # 224/amax_x (half the e4m3fn scale)
            nc.vector.reciprocal(qsx, axb)
            nc.vector.tensor_scalar_mul(qsx, qsx, 224.0)

            # ------------- prefetch block-0 x^T during the collective ------------
            xtt_pre = {}
            for kt in range(N_PRE):
                xtt = xtpool.tile([P, mblk], DT.bfloat16, tag="xtp")
                nc.sync.dma_start(xtt, xs[0:mblk, kt * P:(kt + 1) * P], transpose=True)
                warm_mm(xtt)
                xtt_pre[(0, kt)] = xtt

            # ------------- phase 1b: w abs-max (natural layout) ------------------
            pmax_w = cpool.tile([P, NWT], F32)
            for j in range(NWT):
                wt = natpool.tile([P, d_in], DT.bfloat16, tag="nat")
                nc.scalar.dma_start(wt, w[j * P:(j + 1) * P, :])
                warm_mm(wt)
                nc.vector.tensor_reduce(
                    pmax_w[:, j:j + 1], wt, axis=AX, op=MAX,
                    apply_absolute_value=True)
            lw = cpool.tile([P, 1], F32)
            nc.vector.tensor_reduce(lw, pmax_w, axis=AX, op=MAX)
            awb = cpool.tile([P, 1], F32)
            nc.gpsimd.partition_all_reduce(awb, lw, channels=P,
                                           reduce_op=bass_isa.ReduceOp.max)
            nc.vector.tensor_scalar_max(awb, awb, AMAX_CLAMP)
            qsw = cpool.tile([P, 1], F32)
            nc.vector.reciprocal(qsw, awb)
            nc.vector.tensor_scalar_mul(qsw, qsw, 224.0)
            alpha = cpool.tile([P, 1], F32)  # 4*(ax/448)*(aw/448)
            nc.vector.tensor_mul(alpha, axb, awb)
            nc.vector.tensor_scalar_mul(alpha, alpha, ALPHA_C)

            # ------------- load w transposed, quantize to fp8 slab ---------------
            wq = cpool.tile([P, KT, d_out], FP8)
            for kt in range(KT):
                wtt = wtpool.tile([P, d_out], DT.bfloat16, tag="wtp")
                nc.sync.dma_start(wtt, w[:, kt * P:(kt + 1) * P], transpose=True)
                nc.vector.tensor_scalar_mul(wq[:, kt, :], wtt, qsw)
                warm_mm(wq[:, kt, :], dtype_is_fp8=True)

            # ------------- phase 2: stream x^T, quantize, GEMM -------------------
            for blk in range(NBLK):
                xq = xqpool.tile([P, KT, mblk], FP8, tag="xq")
                for kt in range(KT):
                    xtt = xtt_pre.pop((blk, kt), None)
                    if xtt is None:
                        xtt = xtpool.tile([P, mblk], DT.bfloat16, tag="xtp")
                        nc.sync.dma_start(
                            xtt, xs[blk * mblk:(blk + 1) * mblk, kt * P:(kt + 1) * P],
                            transpose=True)
                    nc.vector.tensor_scalar_mul(xq[:, kt, :], xtt, qsx)
                for mt in range(MSUB):
                    ob = obpool.tile([P, d_out], DT.bfloat16, tag="ob")
                    psums = [pspool.tile([P, NTILE], F32, tag="ps", name=f"ps_{blk}_{mt}_{nt}")
                             for nt in range(NNT)]
                    mlo = mt * P
                    if use_double_row:
                        for kp in range(KT // 2):
                            for nt in range(NNT):
                                nc.tensor.matmul(
                                    psums[nt],
                                    lhsT=xq[:, 2 * kp:2 * kp + 2, mlo:mlo + P],
                                    rhs=wq[:, 2 * kp:2 * kp + 2, nt * NTILE:(nt + 1) * NTILE],
                                    start=(kp == 0), stop=(kp == KT // 2 - 1),
                                    perf_mode=mybir.MatmulPerfMode.DoubleRow)
                    else:
                        for kt in range(KT):
                            for nt in range(NNT):
                                nc.tensor.matmul(
                                    psums[nt],
                                    lhsT=xq[:, kt, mlo:mlo + P],
                                    rhs=wq[:, kt, nt * NTILE:(nt + 1) * NTILE],
                                    start=(kt == 0), stop=(kt == KT - 1))
                    for nt in range(NNT):
                        nc.scalar.mul(ob[:, nt * NTILE:(nt + 1) * NTILE], psums[nt], alpha)
                    nc.scalar.dma_start(out[blk * mblk + mlo:blk * mblk + mlo + P, :], ob)

    nc.finalize()
    _BUILD_CACHE[key] = nc
    return nc


def kernel(x, weight):
    global LAST_RESULT
    from concourse.bass_utils import run_bass_kernel_spmd

    x = np.asarray(x)
    weight = np.asarray(weight)
    if x.dtype != ml_dtypes.bfloat16:
        x = x.astype(ml_dtypes.bfloat16)
    if weight.dtype != ml_dtypes.bfloat16:
        weight = weight.astype(ml_dtypes.bfloat16)
    assert x.shape == (TOKENS, D_IN) and weight.shape == (D_OUT, D_IN)

    tok = TOKENS // N_CORES
    nc = _build(tok, D_IN, D_OUT, N_CORES, USE_DOUBLE_ROW, mblk=2048)

    in_maps = [
        {"xs": np.ascontiguousarray(x[c * tok:(c + 1) * tok]), "w": weight}
        for c in range(N_CORES)
    ]
    res = run_bass_kernel_spmd(nc, in_maps, list(range(N_CORES)), trace=TRACE)
    LAST_RESULT = res
    return np.concatenate([res.results[c]["out"] for c in range(N_CORES)], axis=0)


# revision 12
# speedup vs baseline: 1.0472x; 1.0068x over previous
"""FP8 Linear (dynamic per-tensor e4m3 quantization) on 8 Trainium2 NeuronCores.

Computes the forward value of:
    x_q, s_x = quantize_e4m3fn(x);  w_q, s_w = quantize_e4m3fn(weight)
    out = bf16((x_q*s_x) @ (w_q*s_w).T)        # the bf16 STE shadow GEMM is a
                                               # forward no-op up to bf16 rounding
Strategy per core (data-parallel over tokens, weight replicated):
  1. abs-max over the local x shard (fused pairwise abs_max+reduce on DVE),
     AllReduce(max) across the 8 cores as early as possible; the (replicated)
     weight amax needs no collective and overlaps the AllReduce.
  2. Quantize to the *Trainium* fp8-e4m3 grid at HALF the reference scale
     (TRN e4m3 max normal is 240, not 448; the e4m3fn grid divided by 2 is
     exactly representable, so rounding commutes) and rescale the GEMM output
     by 4*s_x*s_w.
  3. Tiled fp8 GEMM (DoubleRow 2x-pumped), fp32 PSUM accumulation, fused
     scale+bf16-cast on the ScalarE PSUM drain.
  4. Dummy matmuls chained to phase-1 tiles keep the PE HAM clock at 2.4GHz
     through the amax/collective window; x^T DMA-transposes prefetch into
     SBUF during it.
"""

import numpy as np
import ml_dtypes

TOKENS, D_IN, D_OUT = 32768, 2048, 2048
N_CORES = 8
USE_DOUBLE_ROW = True
KEEP_WARM = False

_BUILD_CACHE = {}
LAST_RESULT = None  # BassKernelResults of the most recent device run
TRACE = False       # set True (e.g. from test.py) to capture an NTFF profile


def _build(tok_per_core, d_in, d_out, n_cores, use_double_row, mblk,
           keep_warm=KEEP_WARM):
    key = (tok_per_core, d_in, d_out, n_cores, use_double_row, mblk, keep_warm)
    if key in _BUILD_CACHE:
        return _BUILD_CACHE[key]

    import concourse.bass as bass
    import concourse.mybir as mybir
    import concourse.tile as tile
    from concourse import bacc, bass_isa

    DT = mybir.dt
    P = 128
    NTILE = min(512, d_out)
    assert tok_per_core % mblk == 0 and mblk % P == 0
    assert d_in % P == 0 and d_out % NTILE == 0 and d_out % P == 0
    KT = d_in // P            # k-tiles of 128 along in_features
    NBLK = tok_per_core // mblk
    MSUB = mblk // P          # 128-row m-subtiles per block
    NNT = d_out // NTILE      # n-tiles
    NXT = tok_per_core // P   # natural x tiles for the amax pass
    NWT = d_out // P          # natural w tiles for the amax pass
    WN = min(512, d_in)       # free dim for warm-up matmuls
    if use_double_row:
        assert KT % 2 == 0
    AMAX_CLAMP = 1e-12
    ALPHA_C = float(np.float32(1.0) / np.float32(50176.0))  # 4/448^2

    nc = bacc.Bacc(None, target_bir_lowering=False, num_devices=n_cores)
    xs = nc.dram_tensor("xs", [tok_per_core, d_in], DT.bfloat16, kind="ExternalInput")
    w = nc.dram_tensor("w", [d_out, d_in], DT.bfloat16, kind="ExternalInput")
    out = nc.dram_tensor("out", [tok_per_core, d_out], DT.bfloat16, kind="ExternalOutput")

    F32 = DT.float32
    FP8 = DT.float8e4
    AX = mybir.AxisListType.X
    MAX = mybir.AluOpType.max

    N_PRE = min(12, KT)  # x^T tiles of block 0 prefetched during phase 1

    with tile.TileContext(nc) as tc:
        with (
            tc.tile_pool(name="consts", bufs=1) as cpool,
            tc.tile_pool(name="nat", bufs=4) as natpool,
            tc.tile_pool(name="xtp", bufs=N_PRE) as xtpool,
            tc.tile_pool(name="wtp", bufs=3) as wtpool,
            tc.tile_pool(name="xq", bufs=2) as xqpool,
            tc.tile_pool(name="ob", bufs=2) as obpool,
            tc.tile_pool(name="ps", bufs=7, space="PSUM") as pspool,
            tc.tile_pool(name="warm", bufs=1, space="PSUM") as wmpool,
            tc.tile_pool(name="dram", bufs=1, space="DRAM") as dpool,
        ):
            warm_ps = wmpool.tile([P, NTILE], F32, name="warm_ps") if keep_warm else None

            def warm_mm(src_ap, dtype_is_fp8=False):
                if warm_ps is None:
                    return
                nc.tensor.matmul(warm_ps[:, 0:min(WN, NTILE)],
                                 lhsT=src_ap[:, 0:P],
                                 rhs=src_ap[:, 0:min(WN, NTILE)],
                                 start=True, stop=True)

            # ------------- phase 1a: x abs-max scan (natural layout) -------------
            pmax_x = cpool.tile([P, NXT], F32)
            for i in range(NXT):
                xt = natpool.tile([P, d_in], DT.bfloat16, tag="nat")
                nc.scalar.dma_start(xt, xs[i * P:(i + 1) * P, :])
                warm_mm(xt)
                nc.vector.tensor_reduce(
                    pmax_x[:, i:i + 1], xt, axis=AX, op=MAX,
                    apply_absolute_value=True)

            lx = cpool.tile([P, 1], F32)
            nc.vector.tensor_reduce(lx, pmax_x, axis=AX, op=MAX)
            gx = cpool.tile([P, 1], F32)
            nc.gpsimd.partition_all_reduce(gx, lx, channels=P,
                                           reduce_op=bass_isa.ReduceOp.max)

            # ------------- cross-core AllReduce(max) of the x amax ---------------
            cc_in = dpool.tile([1, 1], F32)
            cc_out = dpool.tile([1, 1], F32,
                                addr_space="Shared" if n_cores > 4 else "Local")
            nc.gpsimd.dma_start(cc_in, gx[0:1, 0:1])
            nc.gpsimd.collective_compute(
                "AllReduce", MAX, replica_groups=[list(range(n_cores))],
                ins=[cc_in.opt()], outs=[cc_out.opt()])
            ax1 = cpool.tile([1, 1], F32)
            nc.gpsimd.dma_start(ax1, cc_out)
            axb = cpool.tile([P, 1], F32)
            nc.gpsimd.partition_broadcast(axb, ax1)
            nc.vector.tensor_scalar_max(axb, axb, AMAX_CLAMP)
            qsx = cpool.tile([P, 1], F32)   # 224/amax_x (half the e4m3fn scale)
            nc.vector.reciprocal(qsx, axb)
            nc.vector.tensor_scalar_mul(qsx, qsx, 224.0)

            # ------------- prefetch block-0 x^T during the collective ------------
            xtt_pre = {}
            for kt in range(N_PRE):
                xtt = xtpool.tile([P, mblk], DT.bfloat16, tag="xtp")
                nc.sync.dma_start(xtt, xs[0:mblk, kt * P:(kt + 1) * P], transpose=True)
                warm_mm(xtt)
                xtt_pre[(0, kt)] = xtt

            # ------------- phase 1b: w abs-max (natural layout) ------------------
            pmax_w = cpool.tile([P, NWT], F32)
            for j in range(NWT):
                wt = natpool.tile([P, d_in], DT.bfloat16, tag="nat")
                nc.scalar.dma_start(wt, w[j * P:(j + 1) * P, :])
                warm_mm(wt)
                nc.vector.tensor_reduce(
                    pmax_w[:, j:j + 1], wt, axis=AX, op=MAX,
                    apply_absolute_value=True)
            lw = cpool.tile([P, 1], F32)
            nc.vector.tensor_reduce(lw, pmax_w, axis=AX, op=MAX)
            awb = cpool.tile([P, 1], F32)
            nc.gpsimd.partition_all_reduce(awb, lw, channels=P,
                                           reduce_op=bass_isa.ReduceOp.max)
            nc.vector.tensor_scalar_max(awb, awb, AMAX_CLAMP)
            qsw = cpool.tile([P, 1], F32)
            nc.vector.reciprocal(qsw, awb)
            nc.vector.tensor_scalar_mul(qsw, qsw, 224.0)
            alpha = cpool.tile([P, 1], F32)  # 4*(ax/448)*(aw/448)
            nc.vector.tensor_mul(alpha, axb, awb)
            nc.vector.tensor_scalar_mul(alpha, alpha, ALPHA_C)

            # ------------- load w transposed, quantize to fp8 slab ---------------
            wq = cpool.tile([P, KT, d_out], FP8)
            for kt in range(KT):
                wtt = wtpool.tile([P, d_out], DT.bfloat16, tag="wtp")
                nc.sync.dma_start(wtt, w[:, kt * P:(kt + 1) * P], transpose=True)
                nc.vector.tensor_scalar_mul(wq[:, kt, :], wtt, qsw)
                warm_mm(wq[:, kt, :], dtype_is_fp8=True)

            # ------------- phase 2: stream x^T, quantize, GEMM -------------------
            for blk in range(NBLK):
                xq = xqpool.tile([P, KT, mblk], FP8, tag="xq")
                for kt in range(KT):
                    xtt = xtt_pre.pop((blk, kt), None)
                    if xtt is None:
                        xtt = xtpool.tile([P, mblk], DT.bfloat16, tag="xtp")
                        nc.sync.dma_start(
                            xtt, xs[blk * mblk:(blk + 1) * mblk, kt * P:(kt + 1) * P],
                            transpose=True)
                    nc.vector.tensor_scalar_mul(xq[:, kt, :], xtt, qsx)
                for mt in range(MSUB):
                    ob = obpool.tile([P, d_out], DT.bfloat16, tag="ob")
                    psums = [pspool.tile([P, NTILE], F32, tag="ps", name=f"ps_{blk}_{mt}_{nt}")
                             for nt in range(NNT)]
                    mlo = mt * P
                    if use_double_row:
                        for kp in range(KT // 2):
                            for nt in range(NNT):
                                nc.tensor.matmul(
                                    psums[nt],
                                    lhsT=xq[:, 2 * kp:2 * kp + 2, mlo:mlo + P],
                                    rhs=wq[:, 2 * kp:2 * kp + 2, nt * NTILE:(nt + 1) * NTILE],
                                    start=(kp == 0), stop=(kp == KT // 2 - 1),
                                    perf_mode=mybir.MatmulPerfMode.DoubleRow)
                    else:
                        for kt in range(KT):
                            for nt in range(NNT):
                                nc.tensor.matmul(
                                    psums[nt],
                                    lhsT=xq[:, kt, mlo:mlo + P],
                                    rhs=wq[:, kt, nt * NTILE:(nt + 1) * NTILE],
                                    start=(kt == 0), stop=(kt == KT - 1))
                    for nt in range(NNT):
                        nc.scalar.mul(ob[:, nt * NTILE:(nt + 1) * NTILE], psums[nt], alpha)
                    nc.scalar.dma_start(out[blk * mblk + mlo:blk * mblk + mlo + P, :], ob)

    nc.finalize()
    _BUILD_CACHE[key] = nc
    return nc


def kernel(x, weight):
    global LAST_RESULT
    from concourse.bass_utils import run_bass_kernel_spmd

    x = np.asarray(x)
    weight = np.asarray(weight)
    if x.dtype != ml_dtypes.bfloat16:
        x = x.astype(ml_dtypes.bfloat16)
    if weight.dtype != ml_dtypes.bfloat16:
        weight = weight.astype(ml_dtypes.bfloat16)
    assert x.shape == (TOKENS, D_IN) and weight.shape == (D_OUT, D_IN)

    tok = TOKENS // N_CORES
    nc = _build(tok, D_IN, D_OUT, N_CORES, USE_DOUBLE_ROW, mblk=2048)

    in_maps = [
        {"xs": np.ascontiguousarray(x[c * tok:(c + 1) * tok]), "w": weight}
        for c in range(N_CORES)
    ]
    res = run_bass_kernel_spmd(nc, in_maps, list(range(N_CORES)), trace=TRACE)
    LAST_RESULT = res
    return np.concatenate([res.results[c]["out"] for c in range(N_CORES)], axis=0)


# revision 14
# speedup vs baseline: 1.0624x; 1.0145x over previous
"""FP8 Linear (dynamic per-tensor e4m3 quantization) on 8 Trainium2 NeuronCores.

Computes the forward value of:
    x_q, s_x = quantize_e4m3fn(x);  w_q, s_w = quantize_e4m3fn(weight)
    out = bf16((x_q*s_x) @ (w_q*s_w).T)        # the bf16 STE shadow GEMM is a
                                               # forward no-op up to bf16 rounding
Per core (data-parallel over tokens, weight replicated):
  1. abs-max over the local x shard on DVE; AllReduce(max) across cores; the
     (replicated) weight's abs-max needs no collective and overlaps it.
  2. While the natural-layout tiles are in SBUF, the idle TensorEngine
     transposes them (identity matmul) into PSUM: block-0 of x is staged as a
     bf16 k-major slab, and the weight is evacuated by ScalarE with the
     quantization scale fused (PSUM -> fp8 slab). This avoids DMA-transpose,
     which hardware-serializes against both plain DMA and collectives.
  3. Quantize to the *Trainium* fp8-e4m3 grid at HALF the reference scale
     (TRN e4m3 max normal is 240, not 448; the e4m3fn grid divided by 2 is
     exactly representable, so rounding commutes) and rescale the GEMM output
     by 4*s_x*s_w.
  4. Tiled fp8 GEMM (DoubleRow 2x-pumped), fp32 PSUM accumulation, fused
     scale+bf16-cast PSUM drains split between ScalarE and VectorE. Block 1's
     x^T streams via DMA-transpose overlapped with the GEMM.
"""

import numpy as np
import ml_dtypes

TOKENS, D_IN, D_OUT = 32768, 2048, 2048
N_CORES = 8
USE_DOUBLE_ROW = True

_BUILD_CACHE = {}
LAST_RESULT = None  # BassKernelResults of the most recent device run
TRACE = False       # set True (e.g. from test.py) to capture an NTFF profile


def _build(tok_per_core, d_in, d_out, n_cores, use_double_row, mblk):
    key = (tok_per_core, d_in, d_out, n_cores, use_double_row, mblk)
    if key in _BUILD_CACHE:
        return _BUILD_CACHE[key]

    import concourse.bass as bass
    import concourse.mybir as mybir
    import concourse.tile as tile
    from concourse import bacc, bass_isa
    from concourse.masks import make_identity

    DT = mybir.dt
    P = 128
    NTILE = min(512, d_out)
    assert tok_per_core % mblk == 0 and mblk % P == 0
    assert d_in % P == 0 and d_out % NTILE == 0 and d_out % P == 0
    KT = d_in // P            # k-tiles of 128 along in_features
    NBLK = tok_per_core // mblk
    MSUB = mblk // P          # 128-row m-subtiles per block
    NNT = d_out // NTILE      # n-tiles
    NXT = tok_per_core // P   # natural x tiles for the amax pass
    NWT = d_out // P          # natural w tiles
    G = min(8, KT)            # k-blocks per PSUM staging bank
    assert KT % G == 0
    NGR = KT // G             # staging banks per natural tile
    MB0 = mblk // P           # token-tiles belonging to block 0
    if use_double_row:
        assert KT % 2 == 0
    AMAX_CLAMP = 1e-12
    ALPHA_C = float(np.float32(1.0) / np.float32(50176.0))  # 4/448^2

    nc = bacc.Bacc(None, target_bir_lowering=False, num_devices=n_cores)
    xs = nc.dram_tensor("xs", [tok_per_core, d_in], DT.bfloat16, kind="ExternalInput")
    w = nc.dram_tensor("w", [d_out, d_in], DT.bfloat16, kind="ExternalInput")
    out = nc.dram_tensor("out", [tok_per_core, d_out], DT.bfloat16, kind="ExternalOutput")

    F32 = DT.float32
    BF16 = DT.bfloat16
    FP8 = DT.float8e4
    AX = mybir.AxisListType.X
    MAX = mybir.AluOpType.max
    Copy = mybir.ActivationFunctionType.Copy

    with tile.TileContext(nc) as tc:
        with (
            tc.tile_pool(name="consts", bufs=1) as cpool,
            tc.tile_pool(name="nat", bufs=4) as natpool,
            tc.tile_pool(name="xtp", bufs=2) as xtpool,
            tc.tile_pool(name="xq", bufs=2) as xqpool,
            tc.tile_pool(name="ob", bufs=2) as obpool,
            tc.tile_pool(name="ps", bufs=6, space="PSUM") as pspool,
            tc.tile_pool(name="tst", bufs=2, space="PSUM") as tstpool,
            tc.tile_pool(name="dram", bufs=1, space="DRAM") as dpool,
        ):
            ident = cpool.tile([P, P], BF16)
            make_identity(nc, ident)
            ones = cpool.tile([1, P], BF16)
            nc.gpsimd.memset(ones, 1.0)

            xT0 = cpool.tile([P, KT, mblk], BF16)   # block-0 x^T, bf16
            wq = cpool.tile([P, KT, d_out], FP8)
            qsw = cpool.tile([P, 1], F32)

            # ---- phase 1a: x scan: abs-max + PE-transpose of block 0 ----------
            pmax_x = cpool.tile([P, NXT], F32)
            for i in range(NXT):
                xt = natpool.tile([P, d_in], BF16, tag="nat", name=f"xt_{i}")
                nc.scalar.dma_start(xt, xs[i * P:(i + 1) * P, :])
                nc.vector.tensor_reduce(
                    pmax_x[:, i:i + 1], xt, axis=AX, op=MAX,
                    apply_absolute_value=True)
                if i < MB0:
                    for g in range(NGR):
                        st = tstpool.tile([P, G * P], BF16, tag="tst",
                                          name=f"stx_{i}_{g}")
                        for jj in range(G):
                            nc.tensor.transpose(
                                st[:, jj * P:(jj + 1) * P],
                                xt[:, (g * G + jj) * P:(g * G + jj + 1) * P], ident)
                        nc.scalar.activation(
                            xT0[:, g * G:(g + 1) * G, i * P:(i + 1) * P],
                            st.rearrange("p (a m) -> p a m", a=G), Copy)

            lx = cpool.tile([P, 1], F32)
            nc.vector.tensor_reduce(lx, pmax_x, axis=AX, op=MAX)
            gx = cpool.tile([P, 1], F32)
            nc.gpsimd.partition_all_reduce(gx, lx, channels=P,
                                           reduce_op=bass_isa.ReduceOp.max)

            # ---- cross-core AllReduce(max) of the x amax ----------------------
            cc_in = dpool.tile([1, 1], F32)
            cc_out = dpool.tile([1, 1], F32,
                                addr_space="Shared" if n_cores > 4 else "Local")
            nc.gpsimd.dma_start(cc_in, gx[0:1, 0:1])
            nc.gpsimd.collective_compute(
                "AllReduce", MAX, replica_groups=[list(range(n_cores))],
                ins=[cc_in.opt()], outs=[cc_out.opt()])
            ax1 = cpool.tile([1, 1], F32)
            nc.gpsimd.dma_start(ax1, cc_out)
            axb = cpool.tile([P, 1], F32)
            nc.gpsimd.partition_broadcast(axb, ax1)
            nc.vector.tensor_scalar_max(axb, axb, AMAX_CLAMP)
            qsx = cpool.tile([P, 1], F32)   # 224/amax_x (half the e4m3fn scale)
            nc.vector.reciprocal(qsx, axb)
            nc.vector.tensor_scalar_mul(qsx, qsx, 224.0)

            # ---- phase 1b: w abs-max scan ------------------------------------
            pmax_w = cpool.tile([P, NWT], F32)
            for j in range(NWT):
                wt = natpool.tile([P, d_in], BF16, tag="nat", name=f"wt_{j}")
                nc.scalar.dma_start(wt, w[j * P:(j + 1) * P, :])
                nc.vector.tensor_reduce(
                    pmax_w[:, j:j + 1], wt, axis=AX, op=MAX,
                    apply_absolute_value=True)

            # w amax partition-reduce without gpsimd (its queue is blocked on
            # the collective): PE-transpose [128,1] -> [1,128], reduce, then
            # broadcast back via a K=1 matmul with ones.
            lw = cpool.tile([P, 1], F32)
            nc.vector.tensor_reduce(lw, pmax_w, axis=AX, op=MAX)
            lwb = cpool.tile([P, 1], BF16)   # amax values are exact in bf16
            nc.vector.tensor_copy(lwb, lw)
            lwt_ps = tstpool.tile([1, P], BF16, tag="tst", name="lwt_ps")
            nc.tensor.transpose(lwt_ps, lwb, ident)
            lwt = cpool.tile([1, P], BF16)
            nc.scalar.copy(lwt, lwt_ps)
            aw1 = cpool.tile([1, 1], BF16)
            nc.vector.tensor_reduce(aw1, lwt, axis=AX, op=MAX)
            awb_ps = tstpool.tile([P, 1], F32, tag="tst", name="awb_ps")
            nc.tensor.matmul(awb_ps, lhsT=ones, rhs=aw1, start=True, stop=True)
            awb = cpool.tile([P, 1], F32)
            nc.scalar.copy(awb, awb_ps)
            nc.vector.tensor_scalar_max(awb, awb, AMAX_CLAMP)
            nc.vector.reciprocal(qsw, awb)
            nc.vector.tensor_scalar_mul(qsw, qsw, 224.0)
            alpha = cpool.tile([P, 1], F32)  # 4*(ax/448)*(aw/448)
            nc.vector.tensor_mul(alpha, axb, awb)
            nc.vector.tensor_scalar_mul(alpha, alpha, ALPHA_C)

            # ---- phase 1c: re-read w, PE-transpose, fused quantize evac -------
            for j in range(NWT):
                wt2 = natpool.tile([P, d_in], BF16, tag="nat", name=f"wt2_{j}")
                nc.scalar.dma_start(wt2, w[j * P:(j + 1) * P, :])
                for g in range(NGR):
                    st = tstpool.tile([P, G * P], BF16, tag="tst",
                                      name=f"stw_{j}_{g}")
                    for jj in range(G):
                        nc.tensor.transpose(
                            st[:, jj * P:(jj + 1) * P],
                            wt2[:, (g * G + jj) * P:(g * G + jj + 1) * P], ident)
                    nc.scalar.activation(
                        wq[:, g * G:(g + 1) * G, j * P:(j + 1) * P],
                        st.rearrange("p (a m) -> p a m", a=G), Copy, scale=qsw)

            # ---- phase 2: quantize x blocks, GEMM ------------------------------
            for blk in range(NBLK):
                xq = xqpool.tile([P, KT, mblk], FP8, tag="xq", name=f"xq_{blk}")
                if blk == 0:
                    for kt in range(KT):
                        nc.vector.tensor_scalar_mul(xq[:, kt, :], xT0[:, kt, :], qsx)
                else:
                    for kt in range(KT):
                        xtt = xtpool.tile([P, mblk], BF16, tag="xtp",
                                          name=f"xtt_{blk}_{kt}")
                        nc.sync.dma_start(
                            xtt, xs[blk * mblk:(blk + 1) * mblk, kt * P:(kt + 1) * P],
                            transpose=True)
                        nc.vector.tensor_scalar_mul(xq[:, kt, :], xtt, qsx)
                for mt in range(MSUB):
                    ob = obpool.tile([P, d_out], BF16, tag="ob", name=f"ob_{blk}_{mt}")
                    psums = [pspool.tile([P, NTILE], F32, tag="ps",
                                         name=f"ps_{blk}_{mt}_{nt}")
                             for nt in range(NNT)]
                    mlo = mt * P
                    if use_double_row:
                        for kp in range(KT // 2):
                            for nt in range(NNT):
                                nc.tensor.matmul(
                                    psums[nt],
                                    lhsT=xq[:, 2 * kp:2 * kp + 2, mlo:mlo + P],
                                    rhs=wq[:, 2 * kp:2 * kp + 2, nt * NTILE:(nt + 1) * NTILE],
                                    start=(kp == 0), stop=(kp == KT // 2 - 1),
                                    perf_mode=mybir.MatmulPerfMode.DoubleRow)
                    else:
                        for kt in range(KT):
                            for nt in range(NNT):
                                nc.tensor.matmul(
                                    psums[nt],
                                    lhsT=xq[:, kt, mlo:mlo + P],
                                    rhs=wq[:, kt, nt * NTILE:(nt + 1) * NTILE],
                                    start=(kt == 0), stop=(kt == KT - 1))
                    for nt in range(NNT):
                        dst = ob[:, nt * NTILE:(nt + 1) * NTILE]
                        if nt % 2 == 0:
                            nc.scalar.mul(dst, psums[nt], alpha)
                        else:
                            nc.vector.tensor_scalar_mul(dst, psums[nt], alpha)
                    nc.scalar.dma_start(out[blk * mblk + mlo:blk * mblk + mlo + P, :], ob)

    nc.finalize()
    _BUILD_CACHE[key] = nc
    return nc


def kernel(x, weight):
    global LAST_RESULT
    from concourse.bass_utils import run_bass_kernel_spmd

    x = np.asarray(x)
    weight = np.asarray(weight)
    if x.dtype != ml_dtypes.bfloat16:
        x = x.astype(ml_dtypes.bfloat16)
    if weight.dtype != ml_dtypes.bfloat16:
        weight = weight.astype(ml_dtypes.bfloat16)
    assert x.shape == (TOKENS, D_IN) and weight.shape == (D_OUT, D_IN)

    tok = TOKENS // N_CORES
    nc = _build(tok, D_IN, D_OUT, N_CORES, USE_DOUBLE_ROW, mblk=2048)

    in_maps = [
        {"xs": np.ascontiguousarray(x[c * tok:(c + 1) * tok]), "w": weight}
        for c in range(N_CORES)
    ]
    res = run_bass_kernel_spmd(nc, in_maps, list(range(N_CORES)), trace=TRACE)
    LAST_RESULT = res
    return np.concatenate([res.results[c]["out"] for c in range(N_CORES)], axis=0)


# revision 15
# speedup vs baseline: 1.1542x; 1.0864x over previous
"""FP8 Linear (dynamic per-tensor e4m3 quantization) on 8 Trainium2 NeuronCores.

Computes the forward value of:
    x_q, s_x = quantize_e4m3fn(x);  w_q, s_w = quantize_e4m3fn(weight)
    out = bf16((x_q*s_x) @ (w_q*s_w).T)        # the bf16 STE shadow GEMM is a
                                               # forward no-op up to bf16 rounding
Per core (data-parallel over tokens, weight replicated):
  1. abs-max over the local x shard on DVE; AllReduce(max) across cores; the
     (replicated) weight's abs-max needs no collective and overlaps it.
  2. While the natural-layout tiles are in SBUF, the idle TensorEngine
     transposes them (identity matmul) into PSUM: block-0 of x is staged as a
     bf16 k-major slab, and the weight is evacuated by ScalarE with the
     quantization scale fused (PSUM -> fp8 slab). This avoids DMA-transpose,
     which hardware-serializes against both plain DMA and collectives.
  3. Quantize to the *Trainium* fp8-e4m3 grid at HALF the reference scale
     (TRN e4m3 max normal is 240, not 448; the e4m3fn grid divided by 2 is
     exactly representable, so rounding commutes) and rescale the GEMM output
     by 4*s_x*s_w.
  4. Tiled fp8 GEMM (DoubleRow 2x-pumped), fp32 PSUM accumulation, fused
     scale+bf16-cast PSUM drains split between ScalarE and VectorE. Block 1's
     x^T streams via DMA-transpose overlapped with the GEMM.
"""

import numpy as np
import ml_dtypes

TOKENS, D_IN, D_OUT = 32768, 2048, 2048
N_CORES = 8
USE_DOUBLE_ROW = True

_BUILD_CACHE = {}
LAST_RESULT = None  # BassKernelResults of the most recent device run
TRACE = False       # set True (e.g. from test.py) to capture an NTFF profile


def _build(tok_per_core, d_in, d_out, n_cores, use_double_row, mblk):
    key = (tok_per_core, d_in, d_out, n_cores, use_double_row, mblk)
    if key in _BUILD_CACHE:
        return _BUILD_CACHE[key]

    import concourse.bass as bass
    import concourse.mybir as mybir
    import concourse.tile as tile
    from concourse import bacc, bass_isa
    from concourse.masks import make_identity

    DT = mybir.dt
    P = 128
    NTILE = min(512, d_out)
    assert tok_per_core % mblk == 0 and mblk % P == 0
    assert d_in % P == 0 and d_out % NTILE == 0 and d_out % P == 0
    KT = d_in // P            # k-tiles of 128 along in_features
    NBLK = tok_per_core // mblk
    MSUB = mblk // P          # 128-row m-subtiles per block
    NNT = d_out // NTILE      # n-tiles
    NXT = tok_per_core // P   # natural x tiles for the amax pass
    NWT = d_out // P          # natural w tiles
    G = min(8, KT)            # k-blocks per PSUM staging bank
    assert KT % G == 0
    NGR = KT // G             # staging banks per natural tile
    MB0 = mblk // P           # token-tiles belonging to block 0
    if use_double_row:
        assert KT % 2 == 0
    AMAX_CLAMP = 1e-12
    ALPHA_C = float(np.float32(1.0) / np.float32(50176.0))  # 4/448^2

    nc = bacc.Bacc(None, target_bir_lowering=False, num_devices=n_cores)
    xs = nc.dram_tensor("xs", [tok_per_core, d_in], DT.bfloat16, kind="ExternalInput")
    w = nc.dram_tensor("w", [d_out, d_in], DT.bfloat16, kind="ExternalInput")
    out = nc.dram_tensor("out", [tok_per_core, d_out], DT.bfloat16, kind="ExternalOutput")

    F32 = DT.float32
    BF16 = DT.bfloat16
    FP8 = DT.float8e4
    AX = mybir.AxisListType.X
    MAX = mybir.AluOpType.max
    Copy = mybir.ActivationFunctionType.Copy

    with tile.TileContext(nc) as tc:
        with (
            tc.tile_pool(name="consts", bufs=1) as cpool,
            tc.tile_pool(name="nat", bufs=4) as natpool,
            tc.tile_pool(name="xtp", bufs=2) as xtpool,
            tc.tile_pool(name="xq", bufs=2) as xqpool,
            tc.tile_pool(name="ob", bufs=2) as obpool,
            tc.tile_pool(name="ps", bufs=6, space="PSUM") as pspool,
            tc.tile_pool(name="tst", bufs=2, space="PSUM") as tstpool,
            tc.tile_pool(name="dram", bufs=1, space="DRAM") as dpool,
        ):
            ident = cpool.tile([P, P], BF16)
            make_identity(nc, ident)
            ones = cpool.tile([1, P], BF16)
            nc.gpsimd.memset(ones, 1.0)

            xT0 = cpool.tile([P, KT, mblk], BF16)   # block-0 x^T, bf16
            wq = cpool.tile([P, KT, d_out], FP8)
            qsw = cpool.tile([P, 1], F32)

            # ---- phase 1b: w abs-max scan ------------------------------------
            pmax_w = cpool.tile([P, NWT], F32)
            for j in range(NWT):
                wt = natpool.tile([P, d_in], BF16, tag="nat", name=f"wt_{j}")
                nc.scalar.dma_start(wt, w[j * P:(j + 1) * P, :])
                nc.vector.tensor_reduce(
                    pmax_w[:, j:j + 1], wt, axis=AX, op=MAX,
                    apply_absolute_value=True)

            # w amax partition-reduce without gpsimd (its queue is blocked on
            # the collective): PE-transpose [128,1] -> [1,128], reduce, then
            # broadcast back via a K=1 matmul with ones.
            lw = cpool.tile([P, 1], F32)
            nc.vector.tensor_reduce(lw, pmax_w, axis=AX, op=MAX)
            lwb = cpool.tile([P, 1], BF16)   # amax values are exact in bf16
            nc.vector.tensor_copy(lwb, lw)
            lwt_ps = tstpool.tile([1, P], BF16, tag="tst", name="lwt_ps")
            nc.tensor.transpose(lwt_ps, lwb, ident)
            lwt = cpool.tile([1, P], BF16)
            nc.scalar.copy(lwt, lwt_ps)
            aw1 = cpool.tile([1, 1], BF16)
            nc.vector.tensor_reduce(aw1, lwt, axis=AX, op=MAX)
            awb_ps = tstpool.tile([P, 1], F32, tag="tst", name="awb_ps")
            nc.tensor.matmul(awb_ps, lhsT=ones, rhs=aw1, start=True, stop=True)
            awb = cpool.tile([P, 1], F32)
            nc.scalar.copy(awb, awb_ps)
            nc.vector.tensor_scalar_max(awb, awb, AMAX_CLAMP)
            nc.vector.reciprocal(qsw, awb)
            nc.vector.tensor_scalar_mul(qsw, qsw, 224.0)

            # ---- phase 1a: x scan: abs-max + PE-transpose of block 0 ----------
            pmax_x = cpool.tile([P, NXT], F32)
            for i in range(NXT):
                xt = natpool.tile([P, d_in], BF16, tag="nat", name=f"xt_{i}")
                nc.scalar.dma_start(xt, xs[i * P:(i + 1) * P, :])
                nc.vector.tensor_reduce(
                    pmax_x[:, i:i + 1], xt, axis=AX, op=MAX,
                    apply_absolute_value=True)
                if i < MB0:
                    for g in range(NGR):
                        st = tstpool.tile([P, G * P], BF16, tag="tst",
                                          name=f"stx_{i}_{g}")
                        for jj in range(G):
                            nc.tensor.transpose(
                                st[:, jj * P:(jj + 1) * P],
                                xt[:, (g * G + jj) * P:(g * G + jj + 1) * P], ident)
                        nc.scalar.activation(
                            xT0[:, g * G:(g + 1) * G, i * P:(i + 1) * P],
                            st.rearrange("p (a m) -> p a m", a=G), Copy)

            lx = cpool.tile([P, 1], F32)
            nc.vector.tensor_reduce(lx, pmax_x, axis=AX, op=MAX)
            gx = cpool.tile([P, 1], F32)
            nc.gpsimd.partition_all_reduce(gx, lx, channels=P,
                                           reduce_op=bass_isa.ReduceOp.max)

            # ---- cross-core AllReduce(max) of the x amax ----------------------
            cc_in = dpool.tile([1, 1], F32)
            cc_out = dpool.tile([1, 1], F32,
                                addr_space="Shared" if n_cores > 4 else "Local")
            nc.gpsimd.dma_start(cc_in, gx[0:1, 0:1])
            nc.gpsimd.collective_compute(
                "AllReduce", MAX, replica_groups=[list(range(n_cores))],
                ins=[cc_in.opt()], outs=[cc_out.opt()])
            ax1 = cpool.tile([1, 1], F32)
            nc.gpsimd.dma_start(ax1, cc_out)
            axb = cpool.tile([P, 1], F32)
            nc.gpsimd.partition_broadcast(axb, ax1)
            nc.vector.tensor_scalar_max(axb, axb, AMAX_CLAMP)
            qsx = cpool.tile([P, 1], F32)   # 224/amax_x (half the e4m3fn scale)
            nc.vector.reciprocal(qsx, axb)
            nc.vector.tensor_scalar_mul(qsx, qsx, 224.0)

            alpha = cpool.tile([P, 1], F32)  # 4*(ax/448)*(aw/448)
            nc.vector.tensor_mul(alpha, axb, awb)
            nc.vector.tensor_scalar_mul(alpha, alpha, ALPHA_C)

            # ---- phase 1c: re-read w, PE-transpose, fused quantize evac -------
            for j in range(NWT):
                wt2 = natpool.tile([P, d_in], BF16, tag="wrt", name=f"wt2_{j}",
                                   bufs=3)
                nc.sync.dma_start(wt2, w[j * P:(j + 1) * P, :])
                for g in range(NGR):
                    st = tstpool.tile([P, G * P], BF16, tag="tst",
                                      name=f"stw_{j}_{g}")
                    for jj in range(G):
                        nc.tensor.transpose(
                            st[:, jj * P:(jj + 1) * P],
                            wt2[:, (g * G + jj) * P:(g * G + jj + 1) * P], ident)
                    nc.scalar.activation(
                        wq[:, g * G:(g + 1) * G, j * P:(j + 1) * P],
                        st.rearrange("p (a m) -> p a m", a=G), Copy, scale=qsw)

            # ---- phase 2: quantize x blocks, GEMM ------------------------------
            for blk in range(NBLK):
                xq = xqpool.tile([P, KT, mblk], FP8, tag="xq", name=f"xq_{blk}")
                if blk == 0:
                    for kt in range(KT):
                        nc.vector.tensor_scalar_mul(xq[:, kt, :], xT0[:, kt, :], qsx)
                else:
                    for kt in range(KT):
                        xtt = xtpool.tile([P, mblk], BF16, tag="xtp",
                                          name=f"xtt_{blk}_{kt}")
                        nc.sync.dma_start(
                            xtt, xs[blk * mblk:(blk + 1) * mblk, kt * P:(kt + 1) * P],
                            transpose=True)
                        nc.vector.tensor_scalar_mul(xq[:, kt, :], xtt, qsx)
                for mt in range(MSUB):
                    ob = obpool.tile([P, d_out], BF16, tag="ob", name=f"ob_{blk}_{mt}")
                    psums = [pspool.tile([P, NTILE], F32, tag="ps",
                                         name=f"ps_{blk}_{mt}_{nt}")
                             for nt in range(NNT)]
                    mlo = mt * P
                    if use_double_row:
                        for kp in range(KT // 2):
                            for nt in range(NNT):
                                nc.tensor.matmul(
                                    psums[nt],
                                    lhsT=xq[:, 2 * kp:2 * kp + 2, mlo:mlo + P],
                                    rhs=wq[:, 2 * kp:2 * kp + 2, nt * NTILE:(nt + 1) * NTILE],
                                    start=(kp == 0), stop=(kp == KT // 2 - 1),
                                    perf_mode=mybir.MatmulPerfMode.DoubleRow)
                    else:
                        for kt in range(KT):
                            for nt in range(NNT):
                                nc.tensor.matmul(
                                    psums[nt],
                                    lhsT=xq[:, kt, mlo:mlo + P],
                                    rhs=wq[:, kt, nt * NTILE:(nt + 1) * NTILE],
                                    start=(kt == 0), stop=(kt == KT - 1))
                    for nt in range(NNT):
                        dst = ob[:, nt * NTILE:(nt + 1) * NTILE]
                        if nt % 2 == 0:
                            nc.scalar.mul(dst, psums[nt], alpha)
                        else:
                            nc.vector.tensor_scalar_mul(dst, psums[nt], alpha)
                    nc.scalar.dma_start(out[blk * mblk + mlo:blk * mblk + mlo + P, :], ob)

    nc.finalize()
    _BUILD_CACHE[key] = nc
    return nc


def kernel(x, weight):
    global LAST_RESULT
    from concourse.bass_utils import run_bass_kernel_spmd

    x = np.asarray(x)
    weight = np.asarray(weight)
    if x.dtype != ml_dtypes.bfloat16:
        x = x.astype(ml_dtypes.bfloat16)
    if weight.dtype != ml_dtypes.bfloat16:
        weight = weight.astype(ml_dtypes.bfloat16)
    assert x.shape == (TOKENS, D_IN) and weight.shape == (D_OUT, D_IN)

    tok = TOKENS // N_CORES
    nc = _build(tok, D_IN, D_OUT, N_CORES, USE_DOUBLE_ROW, mblk=2048)

    in_maps = [
        {"xs": np.ascontiguousarray(x[c * tok:(c + 1) * tok]), "w": weight}
        for c in range(N_CORES)
    ]
    res = run_bass_kernel_spmd(nc, in_maps, list(range(N_CORES)), trace=TRACE)
    LAST_RESULT = res
    return np.concatenate([res.results[c]["out"] for c in range(N_CORES)], axis=0)
